# revision 26
# baseline (speedup 1.0000x reference)
"""Conv2d 3x3 (pad 1, stride 1) + bias on 8 Trainium2 cores.

Problem: x [32,128,56,56] f32, weights [256,128,3,3] f32, bias [256] f32
         -> out [32,256,56,56] f32.

Strategy
--------
Data-parallel over batch: each of the 8 cores owns 4 images.

Per core, implicit GEMM on a shared-padding row layout with stride 57:
  - Padded pixel (ih, iw), ih in [-1,56], iw in [-1,56], lives at flat
    index (ih+1)*57 + (iw+1); the right pad of row r IS the left pad of
    row r+1 (one shared zero column), so the buffer is 58*57+1 = 3307
    floats per channel. The host builds this layout with np.pad.
  - Output is computed in the same stride-57 layout: out position
    p = oh*57 + ow. Every tap (kh, kw) of the 3x3 kernel is then a
    CONSTANT offset kh*57+kw into the flat padded input, so one matmul
    covers 8 output rows at once (N = 8*57 = 456 <= 512 PSUM bank).
  - Weights are host-rearranged to [Cin, (half, tap, co)] so each
    (half, tap) lhsT slice is a contiguous [128,128] stationary tile.
  - 9 taps accumulate into one PSUM tile (start/stop flags); Cout=256 is
    split into 2 halves of 128 partitions.
  - float32r matmuls: full PE rate (1 cycle/row) at N>=256 with ~fp32
    accuracy (fp32 split into bf16 hi/lo inside the PE).
  - Bias is fused into the PSUM->SBUF copy via DVE tensor_scalar_add with
    a per-partition [128,1] scalar.
  - Input DMAs are chunked (weights per tap, images in 4 slices) so the
    first matmuls start as soon as their ranges land — Tile tracks
    dependencies per address range.
  - Host strips the junk column per row at the end.

Built on bacc.Bacc (not raw bass.Bass): walrus engine-instruction structs
hold at most ONE sync wait, and Bacc's compile() runs the
move_matmul_waits_to_ldweights / generate_event_semaphores passes that
split excess waits into EventSemaphore instructions.
"""

import numpy as np

import concourse.bacc as bacc
import concourse.mybir as mybir
import concourse.tile as tile
from concourse.bass_utils import run_bass_kernel_spmd

B, CIN, H, W = 32, 128, 56, 56
COUT = 256
NCORES = 8
BLOC = B // NCORES  # images per core
SP = W + 1  # 57: row stride of the shared-padding layout
# Two trailing zeros (not one): the fp32r matmul free dim must be even,
# so the last tile is a full 456 and its bottom-right tap reads one slot
# past the 58*57+1 layout.
NPIX = (H + 2) * SP + 2  # 3308 padded floats per channel
OUTW = H * SP  # 3192 output cols in stride-57 layout (last one junk)
TILE_N = 8 * SP  # 456: 8 output rows per PSUM tile
NTILES = 7  # 7 x 456 = 3192
LAST_N = TILE_N
# Image DMAs land in group-sized chunks so matmul group t only gates on
# chunk t (group t reads cols < 456*t + 572) and the DMA pipeline always
# leads the PE by a full chunk.
XBOUNDS = [0, 576, 1032, 1488, 1944, 2400, 2856, NPIX]

_nc_cache = None


def _build():
    f32 = mybir.dt.float32
    f32r = mybir.dt.float32r
    nc = bacc.Bacc("TRN2", target_bir_lowering=False)
    x_d = nc.dram_tensor("xp", [BLOC, CIN, NPIX], f32r, kind="ExternalInput")
    w_d = nc.dram_tensor("wT", [CIN, 9 * COUT], f32r, kind="ExternalInput")
    b_d = nc.dram_tensor("bias2", [128, 2], f32, kind="ExternalInput")
    o_d = nc.dram_tensor("out", [BLOC, COUT, OUTW], f32, kind="ExternalOutput")

    with tile.TileContext(nc) as tc:
        with (
            tc.tile_pool(name="wpool", bufs=1) as wpool,
            tc.tile_pool(name="xpool", bufs=2) as xpool,
            tc.tile_pool(name="opool", bufs=4) as opool,
            tc.tile_pool(name="psum", bufs=6, space="PSUM") as psum,
            tc.tile_pool(name="wupsum", bufs=1, space="PSUM") as wupsum,
        ):

            # Weights + bias issue on the ACT HWDGE queue (separate FIFO
            # from the SP queue carrying image chunks), three taps at a
            # time, so the first matmuls gate only on the first chunk +
            # image chunk 0 transferring in parallel. Only image 0 needs
            # fine-grained chunks — later images prefetch a whole image
            # ahead. Keeping the total DMA count down also shortens the
            # kernel-tail semaphore drain.
            wsb = wpool.tile([CIN, 9 * COUT], f32r)
            for tap3 in range(3):
                nc.scalar.dma_start(
                    wsb[:, tap3 * 384 : (tap3 + 1) * 384],
                    w_d[:, tap3 * 384 : (tap3 + 1) * 384],
                )
            bsb = wpool.tile([128, 2], f32)

            # PE warmup: throwaway matmuls against the first weight chunk
            # while the image chunks are still in flight, so the HAM clock
            # gate is already at 8/8 (2.4 GHz) when real matmuls start.
            wup = wupsum.tile([128, 256], f32)
            for _ in range(16):
                nc.tensor.matmul(
                    wup[:], lhsT=wsb[:, :128], rhs=wsb[:, :256],
                    start=True, stop=True,
                )

            for b in range(BLOC):
                xp = xpool.tile([CIN, NPIX], f32r, tag="xp")
                bounds = XBOUNDS if b == 0 else [0, NPIX]
                for lo, hi in zip(bounds, bounds[1:]):
                    nc.sync.dma_start(xp[:, lo:hi], x_d[b, :, lo:hi])
                if b == 0:
                    nc.scalar.dma_start(wsb[:, 9 * 128 :], w_d[:, 9 * 128 :])
                    nc.scalar.dma_start(bsb[:], b_d[:])
                for h in range(2):
                    ot = None
                    for t in range(NTILES):
                        pt = psum.tile([128, TILE_N], f32)
                        for tap in range(9):
                            kh, kw = divmod(tap, 3)
                            off = t * TILE_N + kh * SP + kw
                            c0 = h * (9 * 128) + tap * 128
                            nc.tensor.matmul(
                                pt[:],
                                lhsT=wsb[:, c0 : c0 + 128],
                                rhs=xp[:, off : off + TILE_N],
                                start=(tap == 0),
                                stop=(tap == 8),
                            )
                        # Two groups share one output tile so result
                        # stores go out as one DMA per pair (fewer DMAs =
                        # shorter kernel-tail semaphore drain). The odd
                        # final group ships alone.
                        if t % 2 == 0:
                            ot = opool.tile([128, 2 * TILE_N], f32, tag="ot")
                        half = (t % 2) * TILE_N
                        nc.vector.tensor_scalar_add(
                            ot[:, half : half + TILE_N], pt[:], bsb[:, h : h + 1]
                        )
                        if t % 2 == 1 or t == NTILES - 1:
                            lo = (t - (t % 2)) * TILE_N
                            w_out = (t % 2 + 1) * TILE_N
                            nc.sync.dma_start(
                                o_d[b, h * 128 : (h + 1) * 128, lo : lo + w_out],
                                ot[:, :w_out],
                            )
    nc.compile()
    return nc


def _get_nc():
    global _nc_cache
    if _nc_cache is None:
        _nc_cache = _build()
    return _nc_cache


def _prep_inputs(x, weights, bias):
    x = np.ascontiguousarray(np.asarray(x, dtype=np.float32))
    weights = np.ascontiguousarray(np.asarray(weights, dtype=np.float32))
    bias = np.ascontiguousarray(np.asarray(bias, dtype=np.float32))

    # Shared-padding stride-57 layout: rows -1..56 at stride 57 plus one
    # trailing zero (the last row's right pad).
    xpad = np.pad(x, ((0, 0), (0, 0), (1, 1), (1, 0))).reshape(B, CIN, (H + 2) * SP)
    xp = np.concatenate(
        [xpad, np.zeros((B, CIN, 2), dtype=np.float32)], axis=2
    )  # [B, CIN, 3308]
    # [Cout,Cin,3,3] -> [Cin, (half kh kw co)] so each Cout-half's taps are
    # one contiguous 1152-col block and each lhsT tap slice is contiguous.
    wT = np.ascontiguousarray(
        weights.reshape(2, 128, CIN, 3, 3).transpose(2, 0, 3, 4, 1)
    ).reshape(CIN, 9 * COUT)
    b2 = np.ascontiguousarray(bias.reshape(2, 128).T)  # b2[p, h] = bias[h*128+p]

    return [
        {
            "xp": np.ascontiguousarray(xp[i * BLOC : (i + 1) * BLOC]),
            "wT": wT,
            "bias2": b2,
        }
        for i in range(NCORES)
    ]


def _run(inputs, trace=False):
    in_maps = _prep_inputs(inputs["x"], inputs["weights"], inputs["bias"])
    res = run_bass_kernel_spmd(
        _get_nc(), in_maps, core_ids=list(range(NCORES)), trace=trace
    )
    out = np.concatenate([r["out"] for r in res.results], axis=0)  # [B, COUT, 3192]
    out = out.reshape(B, COUT, H, SP)[:, :, :, :W]
    return np.ascontiguousarray(out), res


def kernel(x, weights, bias):
    out, _ = _run({"x": x, "weights": weights, "bias": bias})
    return out


# revision 27
# speedup vs baseline: 1.0048x; 1.0048x over previous
"""Conv2d 3x3 (pad 1, stride 1) + bias on 8 Trainium2 cores.

Problem: x [32,128,56,56] f32, weights [256,128,3,3] f32, bias [256] f32
         -> out [32,256,56,56] f32.

Strategy
--------
Data-parallel over batch: each of the 8 cores owns 4 images.

Per core, implicit GEMM on a shared-padding row layout with stride 57:
  - Padded pixel (ih, iw), ih in [-1,56], iw in [-1,56], lives at flat
    index (ih+1)*57 + (iw+1); the right pad of row r IS the left pad of
    row r+1 (one shared zero column), so the buffer is 58*57+1 = 3307
    floats per channel. The host builds this layout with np.pad.
  - Output is computed in the same stride-57 layout: out position
    p = oh*57 + ow. Every tap (kh, kw) of the 3x3 kernel is then a
    CONSTANT offset kh*57+kw into the flat padded input, so one matmul
    covers 8 output rows at once (N = 8*57 = 456 <= 512 PSUM bank).
  - Weights are host-rearranged to [Cin, (half, tap, co)] so each
    (half, tap) lhsT slice is a contiguous [128,128] stationary tile.
  - 9 taps accumulate into one PSUM tile (start/stop flags); Cout=256 is
    split into 2 halves of 128 partitions.
  - float32r matmuls: full PE rate (1 cycle/row) at N>=256 with ~fp32
    accuracy (fp32 split into bf16 hi/lo inside the PE).
  - Bias is fused into the PSUM->SBUF copy via DVE tensor_scalar_add with
    a per-partition [128,1] scalar.
  - Input DMAs are chunked (weights per tap, images in 4 slices) so the
    first matmuls start as soon as their ranges land — Tile tracks
    dependencies per address range.
  - Host strips the junk column per row at the end.

Built on bacc.Bacc (not raw bass.Bass): walrus engine-instruction structs
hold at most ONE sync wait, and Bacc's compile() runs the
move_matmul_waits_to_ldweights / generate_event_semaphores passes that
split excess waits into EventSemaphore instructions.
"""

import numpy as np

import concourse.bacc as bacc
import concourse.mybir as mybir
import concourse.tile as tile
from concourse.bass_utils import run_bass_kernel_spmd

B, CIN, H, W = 32, 128, 56, 56
COUT = 256
NCORES = 8
BLOC = B // NCORES  # images per core
SP = W + 1  # 57: row stride of the shared-padding layout
# Two trailing zeros (not one): the fp32r matmul free dim must be even,
# so the last tile is a full 456 and its bottom-right tap reads one slot
# past the 58*57+1 layout.
NPIX = (H + 2) * SP + 2  # 3308 padded floats per channel
OUTW = H * SP  # 3192 output cols in stride-57 layout (last one junk)
TILE_N = 8 * SP  # 456: 8 output rows per PSUM tile
NTILES = 7  # 7 x 456 = 3192
LAST_N = TILE_N
# Image DMAs land in group-sized chunks so matmul group t only gates on
# chunk t (group t reads cols < 456*t + 572) and the DMA pipeline always
# leads the PE by a full chunk.
XBOUNDS = [0, 576, 1032, 1488, 1944, 2400, 2856, NPIX]

_nc_cache = None


def _build():
    f32 = mybir.dt.float32
    f32r = mybir.dt.float32r
    nc = bacc.Bacc("TRN2", target_bir_lowering=False)
    x_d = nc.dram_tensor("xp", [BLOC, CIN, NPIX], f32r, kind="ExternalInput")
    w_d = nc.dram_tensor("wT", [CIN, 9 * COUT], f32r, kind="ExternalInput")
    b_d = nc.dram_tensor("bias2", [128, 2], f32, kind="ExternalInput")
    o_d = nc.dram_tensor("out", [BLOC, COUT, OUTW], f32, kind="ExternalOutput")

    with tile.TileContext(nc) as tc:
        with (
            tc.tile_pool(name="wpool", bufs=1) as wpool,
            tc.tile_pool(name="xpool", bufs=2) as xpool,
            tc.tile_pool(name="opool", bufs=4) as opool,
            tc.tile_pool(name="psum", bufs=6, space="PSUM") as psum,
            tc.tile_pool(name="wupsum", bufs=1, space="PSUM") as wupsum,
        ):

            # Weights + bias issue on the ACT HWDGE queue (separate FIFO
            # from the SP queue carrying image chunks), three taps at a
            # time, so the first matmuls gate only on the first chunk +
            # image chunk 0 transferring in parallel. Only image 0 needs
            # fine-grained chunks — later images prefetch a whole image
            # ahead. Keeping the total DMA count down also shortens the
            # kernel-tail semaphore drain.
            wsb = wpool.tile([CIN, 9 * COUT], f32r)
            for tap3 in range(3):
                nc.scalar.dma_start(
                    wsb[:, tap3 * 384 : (tap3 + 1) * 384],
                    w_d[:, tap3 * 384 : (tap3 + 1) * 384],
                )
            bsb = wpool.tile([128, 2], f32)

            # PE warmup: bf16 throwaway matmuls on a memset tile (no DMA
            # dependency, so they start during the preamble) sized to run
            # until the first input chunks land — the HAM clock gate is
            # then already at 8/8 (2.4 GHz) when real matmuls start, and
            # the PE never idles long enough to re-throttle.
            wub = wpool.tile([128, 512], mybir.dt.bfloat16)
            nc.vector.memset(wub[:], 0.0)
            wup = wupsum.tile([128, 512], f32)
            for _ in range(28):
                nc.tensor.matmul(
                    wup[:], lhsT=wub[:, :128], rhs=wub[:],
                    start=True, stop=True,
                )

            for b in range(BLOC):
                xp = xpool.tile([CIN, NPIX], f32r, tag="xp")
                bounds = XBOUNDS if b == 0 else [0, NPIX]
                for lo, hi in zip(bounds, bounds[1:]):
                    nc.sync.dma_start(xp[:, lo:hi], x_d[b, :, lo:hi])
                if b == 0:
                    nc.scalar.dma_start(wsb[:, 9 * 128 :], w_d[:, 9 * 128 :])
                    nc.scalar.dma_start(bsb[:], b_d[:])
                for h in range(2):
                    ot = None
                    for t in range(NTILES):
                        pt = psum.tile([128, TILE_N], f32)
                        for tap in range(9):
                            kh, kw = divmod(tap, 3)
                            off = t * TILE_N + kh * SP + kw
                            c0 = h * (9 * 128) + tap * 128
                            nc.tensor.matmul(
                                pt[:],
                                lhsT=wsb[:, c0 : c0 + 128],
                                rhs=xp[:, off : off + TILE_N],
                                start=(tap == 0),
                                stop=(tap == 8),
                            )
                        # Two groups share one output tile so result
                        # stores go out as one DMA per pair (fewer DMAs =
                        # shorter kernel-tail semaphore drain). The odd
                        # final group ships alone.
                        if t % 2 == 0:
                            ot = opool.tile([128, 2 * TILE_N], f32, tag="ot")
                        half = (t % 2) * TILE_N
                        nc.vector.tensor_scalar_add(
                            ot[:, half : half + TILE_N], pt[:], bsb[:, h : h + 1]
                        )
                        if t % 2 == 1 or t == NTILES - 1:
                            lo = (t - (t % 2)) * TILE_N
                            w_out = (t % 2 + 1) * TILE_N
                            nc.sync.dma_start(
                                o_d[b, h * 128 : (h + 1) * 128, lo : lo + w_out],
                                ot[:, :w_out],
                            )
    nc.compile()
    return nc


def _get_nc():
    global _nc_cache
    if _nc_cache is None:
        _nc_cache = _build()
    return _nc_cache


def _prep_inputs(x, weights, bias):
    x = np.ascontiguousarray(np.asarray(x, dtype=np.float32))
    weights = np.ascontiguousarray(np.asarray(weights, dtype=np.float32))
    bias = np.ascontiguousarray(np.asarray(bias, dtype=np.float32))

    # Shared-padding stride-57 layout: rows -1..56 at stride 57 plus one
    # trailing zero (the last row's right pad).
    xpad = np.pad(x, ((0, 0), (0, 0), (1, 1), (1, 0))).reshape(B, CIN, (H + 2) * SP)
    xp = np.concatenate(
        [xpad, np.zeros((B, CIN, 2), dtype=np.float32)], axis=2
    )  # [B, CIN, 3308]
    # [Cout,Cin,3,3] -> [Cin, (half kh kw co)] so each Cout-half's taps are
    # one contiguous 1152-col block and each lhsT tap slice is contiguous.
    wT = np.ascontiguousarray(
        weights.reshape(2, 128, CIN, 3, 3).transpose(2, 0, 3, 4, 1)
    ).reshape(CIN, 9 * COUT)
    b2 = np.ascontiguousarray(bias.reshape(2, 128).T)  # b2[p, h] = bias[h*128+p]

    return [
        {
            "xp": np.ascontiguousarray(xp[i * BLOC : (i + 1) * BLOC]),
            "wT": wT,
            "bias2": b2,
        }
        for i in range(NCORES)
    ]


def _run(inputs, trace=False):
    in_maps = _prep_inputs(inputs["x"], inputs["weights"], inputs["bias"])
    res = run_bass_kernel_spmd(
        _get_nc(), in_maps, core_ids=list(range(NCORES)), trace=trace
    )
    out = np.concatenate([r["out"] for r in res.results], axis=0)  # [B, COUT, 3192]
    out = out.reshape(B, COUT, H, SP)[:, :, :, :W]
    return np.ascontiguousarray(out), res


def kernel(x, weights, bias):
    out, _ = _run({"x": x, "weights": weights, "bias": bias})
    return out


# revision 30
# speedup vs baseline: 1.0386x; 1.0336x over previous
"""Conv2d 3x3 (pad 1, stride 1) + bias on 8 Trainium2 cores.

Problem: x [32,128,56,56] f32, weights [256,128,3,3] f32, bias [256] f32
         -> out [32,256,56,56] f32.

Strategy
--------
Data-parallel over batch: each of the 8 cores owns 4 images.

Per core, implicit GEMM on a shared-padding row layout with stride 57:
  - Padded pixel (ih, iw), ih in [-1,56], iw in [-1,56], lives at flat
    index (ih+1)*57 + (iw+1); the right pad of row r IS the left pad of
    row r+1 (one shared zero column), so the buffer is 58*57+1 = 3307
    floats per channel. The host builds this layout with np.pad.
  - Output is computed in the same stride-57 layout: out position
    p = oh*57 + ow. Every tap (kh, kw) of the 3x3 kernel is then a
    CONSTANT offset kh*57+kw into the flat padded input, so one matmul
    covers 8 output rows at once (N = 8*57 = 456 <= 512 PSUM bank).
  - Weights are host-rearranged to [Cin, (half, tap, co)] so each
    (half, tap) lhsT slice is a contiguous [128,128] stationary tile.
  - 9 taps accumulate into one PSUM tile (start/stop flags); Cout=256 is
    split into 2 halves of 128 partitions.
  - float32r matmuls: full PE rate (1 cycle/row) at N>=256 with ~fp32
    accuracy (fp32 split into bf16 hi/lo inside the PE).
  - Bias is fused into the PSUM->SBUF copy via DVE tensor_scalar_add with
    a per-partition [128,1] scalar.
  - Input DMAs are chunked (weights per tap, images in 4 slices) so the
    first matmuls start as soon as their ranges land — Tile tracks
    dependencies per address range.
  - Host strips the junk column per row at the end.

Built on bacc.Bacc (not raw bass.Bass): walrus engine-instruction structs
hold at most ONE sync wait, and Bacc's compile() runs the
move_matmul_waits_to_ldweights / generate_event_semaphores passes that
split excess waits into EventSemaphore instructions.
"""

import numpy as np

import concourse.bacc as bacc
import concourse.mybir as mybir
import concourse.tile as tile
from concourse.bass_utils import run_bass_kernel_spmd

B, CIN, H, W = 32, 128, 56, 56
COUT = 256
NCORES = 8
BLOC = B // NCORES  # images per core
SP = W + 1  # 57: row stride of the shared-padding layout
# Two trailing zeros (not one): the fp32r matmul free dim must be even,
# so the last tile is a full 456 and its bottom-right tap reads one slot
# past the 58*57+1 layout.
NPIX = (H + 2) * SP + 2  # 3308 padded floats per channel
OUTW = H * SP  # 3192 output cols in stride-57 layout (last one junk)
TILE_N = 8 * SP  # 456: 8 output rows per PSUM tile
NTILES = 7  # 7 x 456 = 3192
LAST_N = TILE_N
# Image DMAs land in group-sized chunks so matmul group t only gates on
# chunk t (group t reads cols < 456*t + 572) and the DMA pipeline always
# leads the PE by a full chunk.
XBOUNDS = [0, 576, 1032, 1488, 1944, 2400, 2856, NPIX]

_nc_cache = None


def _build():
    f32 = mybir.dt.float32
    f32r = mybir.dt.float32r
    nc = bacc.Bacc("TRN2", target_bir_lowering=False)
    x_d = nc.dram_tensor("xp", [BLOC, CIN, NPIX], f32r, kind="ExternalInput")
    w_d = nc.dram_tensor("wT", [CIN, 9 * COUT], f32r, kind="ExternalInput")
    b_d = nc.dram_tensor("bias2", [128, 2], f32, kind="ExternalInput")
    o_d = nc.dram_tensor("out", [BLOC, COUT, OUTW], f32, kind="ExternalOutput")

    with tile.TileContext(nc) as tc:
        with (
            tc.tile_pool(name="wpool", bufs=1) as wpool,
            tc.tile_pool(name="xpool", bufs=2) as xpool,
            tc.tile_pool(name="opool", bufs=4) as opool,
            tc.tile_pool(name="psum", bufs=6, space="PSUM") as psum,
            tc.tile_pool(name="wupsum", bufs=1, space="PSUM") as wupsum,
        ):

            # Weights + bias issue on the ACT HWDGE queue (separate FIFO
            # from the SP queue carrying image chunks), three taps at a
            # time, so the first matmuls gate only on the first chunk +
            # image chunk 0 transferring in parallel. Only image 0 needs
            # fine-grained chunks — later images prefetch a whole image
            # ahead. Keeping the total DMA count down also shortens the
            # kernel-tail semaphore drain.
            wsb = wpool.tile([CIN, 9 * COUT], f32r)
            for tap3 in range(3):
                nc.scalar.dma_start(
                    wsb[:, tap3 * 384 : (tap3 + 1) * 384],
                    w_d[:, tap3 * 384 : (tap3 + 1) * 384],
                )
            bsb = wpool.tile([128, 2], f32)

            # PE warmup: bf16 throwaway matmuls on a memset tile (no DMA
            # dependency, so they start during the preamble) sized to run
            # until the first input chunks land — the HAM clock gate is
            # then already at 8/8 (2.4 GHz) when real matmuls start, and
            # the PE never idles long enough to re-throttle.
            wub = wpool.tile([128, 512], mybir.dt.bfloat16)
            nc.vector.memset(wub[:], 0.0)
            wup = wupsum.tile([128, 512], f32)
            for _ in range(8):
                nc.tensor.matmul(
                    wup[:], lhsT=wub[:, :128], rhs=wub[:],
                    start=True, stop=True,
                )

            for b in range(BLOC):
                xp = xpool.tile([CIN, NPIX], f32r, tag="xp")
                bounds = XBOUNDS if b == 0 else [0, NPIX]
                for lo, hi in zip(bounds, bounds[1:]):
                    nc.sync.dma_start(xp[:, lo:hi], x_d[b, :, lo:hi])
                if b == 0:
                    nc.scalar.dma_start(wsb[:, 9 * 128 :], w_d[:, 9 * 128 :])
                    nc.scalar.dma_start(bsb[:], b_d[:])
                for h in range(2):
                    # Tiles are processed in pairs with their taps
                    # interleaved: consecutive matmuls alternate PSUM
                    # banks and reuse the same stationary weights, hiding
                    # more of the per-matmul weight-load overhead. Each
                    # pair ships as one output DMA (fewer DMAs = shorter
                    # kernel-tail semaphore drain); the odd final group
                    # ships alone.
                    for t0 in range(0, NTILES, 2):
                        ts_ = [t0] if t0 == NTILES - 1 else [t0, t0 + 1]
                        pts = [
                            psum.tile([128, TILE_N], f32, tag="pt", name=f"pt{k}")
                            for k in range(len(ts_))
                        ]
                        for tap in range(9):
                            kh, kw = divmod(tap, 3)
                            c0 = h * (9 * 128) + tap * 128
                            for pt, t in zip(pts, ts_):
                                off = t * TILE_N + kh * SP + kw
                                nc.tensor.matmul(
                                    pt[:],
                                    lhsT=wsb[:, c0 : c0 + 128],
                                    rhs=xp[:, off : off + TILE_N],
                                    start=(tap == 0),
                                    stop=(tap == 8),
                                )
                        ot = opool.tile([128, 2 * TILE_N], f32, tag="ot")
                        for k, (pt, t) in enumerate(zip(pts, ts_)):
                            nc.vector.tensor_scalar_add(
                                ot[:, k * TILE_N : (k + 1) * TILE_N],
                                pt[:],
                                bsb[:, h : h + 1],
                            )
                        w_out = len(ts_) * TILE_N
                        nc.sync.dma_start(
                            o_d[b, h * 128 : (h + 1) * 128,
                                t0 * TILE_N : t0 * TILE_N + w_out],
                            ot[:, :w_out],
                        )
    nc.compile()
    return nc


def _get_nc():
    global _nc_cache
    if _nc_cache is None:
        _nc_cache = _build()
    return _nc_cache


def _prep_inputs(x, weights, bias):
    x = np.ascontiguousarray(np.asarray(x, dtype=np.float32))
    weights = np.ascontiguousarray(np.asarray(weights, dtype=np.float32))
    bias = np.ascontiguousarray(np.asarray(bias, dtype=np.float32))

    # Shared-padding stride-57 layout: rows -1..56 at stride 57 plus one
    # trailing zero (the last row's right pad).
    xpad = np.pad(x, ((0, 0), (0, 0), (1, 1), (1, 0))).reshape(B, CIN, (H + 2) * SP)
    xp = np.concatenate(
        [xpad, np.zeros((B, CIN, 2), dtype=np.float32)], axis=2
    )  # [B, CIN, 3308]
    # [Cout,Cin,3,3] -> [Cin, (half kh kw co)] so each Cout-half's taps are
    # one contiguous 1152-col block and each lhsT tap slice is contiguous.
    wT = np.ascontiguousarray(
        weights.reshape(2, 128, CIN, 3, 3).transpose(2, 0, 3, 4, 1)
    ).reshape(CIN, 9 * COUT)
    b2 = np.ascontiguousarray(bias.reshape(2, 128).T)  # b2[p, h] = bias[h*128+p]

    return [
        {
            "xp": np.ascontiguousarray(xp[i * BLOC : (i + 1) * BLOC]),
            "wT": wT,
            "bias2": b2,
        }
        for i in range(NCORES)
    ]


def _run(inputs, trace=False):
    in_maps = _prep_inputs(inputs["x"], inputs["weights"], inputs["bias"])
    res = run_bass_kernel_spmd(
        _get_nc(), in_maps, core_ids=list(range(NCORES)), trace=trace
    )
    out = np.concatenate([r["out"] for r in res.results], axis=0)  # [B, COUT, 3192]
    out = out.reshape(B, COUT, H, SP)[:, :, :, :W]
    return np.ascontiguousarray(out), res


def kernel(x, weights, bias):
    out, _ = _run({"x": x, "weights": weights, "bias": bias})
    return out
